# revision 17
# baseline (speedup 1.0000x reference)
"""DigitCaps dynamic-routing kernel v2.1 for 8x TRN2 NeuronCores (Bass/Tile).

DigitCaps routing kernel. Design notes:
  - T1 PSUM in two halves [i-pair, (h,448-in-512)], drained on ACT, mul/adds
    per half on DVE (all bf16 2x mode).
  - iter-2 T1 runs on the UNSQUASHED s1 (squash scale g1 is folded into the
    iter-2 logits via one fused scalar_tensor_tensor: a = a2_raw*g1 + a1).
    This removes the per-j Sqrt (ACT table thrash) from the critical path.
  - squash scalars (1/Z, q, sqrt, g) batched per group of 7 capsules on
    [128, 7] tiles; only ~2 activation-table swaps per iteration.
  - Z comes free from exp's accum_out (pad-exact: padded logits are 0).
  - W2 streaming DMAs issue from the gpsimd queue so e-transposes on the
    sync queue never block prefetch.
"""

import sys

sys.path.insert(0, "/opt/trn_rl_repo")

import numpy as np
import ml_dtypes
from contextlib import ExitStack

import concourse.bacc as bacc
import concourse.bass as bass
import concourse.tile as tile
from concourse import mybir
from concourse.masks import make_identity
from concourse.bass_utils import run_bass_kernel_spmd

F32 = mybir.dt.float32
BF16 = mybir.dt.bfloat16
AX = mybir.AxisListType
ACT_F = mybir.ActivationFunctionType
OP = mybir.AluOpType

B, J, R, O, I = 128, 166, 864, 8, 4
NCORES = 8
JL = 21
RP = 896
H = 448
C32 = 27
C128 = 7
JO = JL * O
JB = JL * B
NPAD = RP - R
GRP = 21         # squash-scalar batch size (JL must divide evenly)
RSQRT_K = 1.26653360130029e19  # bitcast(bits(q)>>1)*K ~= sqrt(q), +-3%


def _dve_sqrt(nc, pool, out_ap, q_ap, n, tag="dsq"):
    """out = sqrt(q) computed entirely on DVE (shift-seed + exact recip +
    2 Newton rsqrt steps; ~2e-6 rel err). Avoids ACT Sqrt, whose table set
    differs from Exp's and would force two ~1.3us table reloads right in
    the middle of the routing-exp stream."""
    U32 = mybir.dt.uint32
    s = pool.tile([128, n], F32, tag=tag + "s", name=tag + "s")
    nc.vector.tensor_scalar(
        s[:].bitcast(U32), q_ap.bitcast(U32), 1, None, OP.logical_shift_right
    )
    nc.vector.tensor_scalar_mul(s[:], s[:], RSQRT_K)
    y = pool.tile([128, n], F32, tag=tag + "y", name=tag + "y")
    nc.vector.reciprocal(y[:], s[:])
    t = pool.tile([128, n], F32, tag=tag + "t", name=tag + "t")
    for _ in range(2):
        nc.vector.tensor_mul(t[:], y[:], y[:])
        nc.vector.tensor_mul(t[:], t[:], q_ap)
        nc.vector.tensor_scalar(t[:], t[:], -0.5, 1.5, OP.mult, OP.add)
        nc.vector.tensor_mul(y[:], y[:], t[:])
    nc.vector.tensor_mul(out_ap, y[:], q_ap)


def build_nc(repeat=1):
    nc = bacc.Bacc(
        "TRN2", target_bir_lowering=False, debug=False, enable_asserts=False
    )

    d_Wr = nc.dram_tensor("Wr", [128, C128 * I * JL * O], BF16, kind="ExternalInput")
    d_W2 = nc.dram_tensor("W2", [JL, 8, I * RP], BF16, kind="ExternalInput")
    d_u_b = nc.dram_tensor("u_b", [128, I * RP], BF16, kind="ExternalInput")
    d_uTr = nc.dram_tensor("uTr", [128, C128 * I * B], BF16, kind="ExternalInput")
    d_out = nc.dram_tensor("v_out", [128, JL * O], F32, kind="ExternalOutput")

    with tile.TileContext(nc) as tc:
        for _ in range(repeat):
            _body(tc, d_Wr, d_W2, d_u_b, d_uTr, d_out)
    nc.compile()
    return nc


def _squash_b_layout(nc, pool, s0_sb, v0b):
    sqb = pool.tile([128, JO], F32, name="sqb")
    nc.vector.tensor_mul(sqb[:], s0_sb[:], s0_sb[:])
    n2b = pool.tile([128, JL], F32, name="n2b")
    nc.vector.reduce_sum(
        n2b[:], sqb[:].rearrange("p (j o) -> p j o", j=JL), axis=AX.X
    )
    rtb = pool.tile([128, JL], F32, name="rtb")
    _dve_sqrt(nc, pool, rtb[:], n2b[:], JL, tag="s0sq")
    nc.vector.tensor_scalar_add(n2b[:], n2b[:], 1.0)
    nc.vector.reciprocal(n2b[:], n2b[:])
    nc.vector.tensor_mul(rtb[:], rtb[:], n2b[:])
    nc.vector.tensor_mul(
        v0b[:].rearrange("p (j o) -> p j o", j=JL),
        s0_sb[:].rearrange("p (j o) -> p j o", j=JL),
        rtb[:].unsqueeze(2).broadcast_to([128, JL, O]),
    )


def _body(tc, d_Wr, d_W2, d_u_b, d_uTr, d_out):
    nc = tc.nc
    es = ExitStack()
    const = es.enter_context(tc.tile_pool(name="const", bufs=1))
    stream = es.enter_context(tc.tile_pool(name="stream", bufs=5))
    work = es.enter_context(tc.tile_pool(name="work", bufs=3))
    ework = es.enter_context(tc.tile_pool(name="ework", bufs=5))
    small = es.enter_context(tc.tile_pool(name="small", bufs=6))
    scal = es.enter_context(tc.tile_pool(name="scal", bufs=2))

    with es:
        # ---------------- constants / persistent loads ----------------
        identity = const.tile([128, 128], F32)
        make_identity(nc, identity[:])

        u_b = const.tile([128, I * RP], BF16)
        uTr_all = const.tile([128, C128 * I * B], BF16)
        uTr_v = uTr_all[:].rearrange("p (c i b) -> p c i b", c=C128, i=I)
        wr_all = const.tile([128, C128 * I * JL * O], BF16)
        CIB = I * B
        CW = I * JL * O
        # uTr on sync queue, wr on scalar queue: two HWDGE queues in parallel
        # halve the serial prologue load; subtile deps let s0's matmuls start
        # per-chunk as each DMA lands.
        for c in range(C128):
            nc.sync.dma_start(
                uTr_all[:, c * CIB : (c + 1) * CIB], d_uTr[:, c * CIB : (c + 1) * CIB]
            )
            nc.scalar.dma_start(
                wr_all[:, c * CW : (c + 1) * CW], d_Wr[:, c * CW : (c + 1) * CW]
            )
        nc.gpsimd.dma_start(u_b[:], d_u_b[:])
        wr_v = wr_all[:].rearrange(
            "p (c i j o) -> p c i j o", c=C128, i=I, j=JL
        )

        a1_all = const.tile([128, JL * RP], BF16)
        a1_v = a1_all[:].rearrange("p (j r) -> p j r", j=JL)
        sT_prev = const.tile([8, JB], BF16)       # transposed (un)squashed s
        out_sb = const.tile([128, JL * O], F32)
        s1_all = const.tile([128, JL * O], F32)   # iter-1 raw s per j
        s2_all = const.tile([128, JL * O], F32)   # iter-2 raw s per j

        # ---------------- s0 from resident wr/uTr (uniform routing) -------
        with tc.tile_pool(name="s0p", bufs=2) as s0p, tc.tile_pool(
            name="s0ps_pool", bufs=1, space="PSUM"
        ) as s0psp:
            s0ps = s0psp.tile([128, 512], F32, tag="s0ps", name="s0ps")
            wr_j = wr_all[:].rearrange("p (c i n) -> p c i n", c=C128, i=I)
            for c in range(C128):
                for i in range(I):
                    nc.tensor.matmul(
                        s0ps[:, :JO],
                        lhsT=uTr_v[:, c, i],
                        rhs=wr_j[:, c, i],
                        start=(c == 0 and i == 0),
                        stop=(c == C128 - 1 and i == I - 1),
                    )
            s0_sb = s0p.tile([128, JO], F32, name="s0_sb")
            nc.scalar.activation(s0_sb[:], s0ps[:, :JO], ACT_F.Copy, scale=1.0 / R)
            v0b = s0p.tile([128, JO], F32, name="v0b")
            _squash_b_layout(nc, s0p, s0_sb, v0b)
            jj = 0
            while jj < JL:
                take = min(4, JL - jj)
                tps = s0psp.tile([128, 512], F32, tag="s0ps", name="tps")
                for q in range(take):
                    nc.tensor.transpose(
                        tps[0:8, q * 128 : (q + 1) * 128],
                        v0b[:].rearrange("p (j o) -> p j o", j=JL)[:, jj + q, :],
                        identity[:],
                    )
                nc.scalar.copy(
                    sT_prev[:, jj * 128 : (jj + take) * 128], tps[0:8, : take * 128]
                )
                jj += take

        # ---------------- routing iterations, per-j pipelined ----------------
        with tc.tile_pool(name="psumT1", bufs=2, space="PSUM") as psumT1, \
             tc.tile_pool(name="psumSV", bufs=4, space="PSUM") as psumSV:
            for t in (1, 2):
                # per-iteration scalar stores [128, JL]
                zraw_all = scal.tile([128, JL], F32, tag="zraw", name=f"zraw{t}")
                m2_all = scal.tile([128, JL], F32, tag="m2", name=f"m2{t}")
                zr_all = scal.tile([128, JL], F32, tag="zrall", name=f"zr{t}")
                g_all = scal.tile([128, JL], F32, tag="gall", name=f"g{t}")
                if t == 1:
                    g1_all = g_all

                eT = {}
                curs = {}
                spss = {}
                w2pairs = {}

                def stageA(j, t=t, eT=eT):
                    w2 = stream.tile([8, I * RP], BF16, tag="w2", name=f"w2_{t}_{j}")
                    nc.gpsimd.dma_start(w2[:], d_W2[j])
                    w2v = w2[:]
                    lhs = sT_prev[:, j * 128 : (j + 1) * 128]

                    # T1 quarters: per i, 2 mm of n=448, quarter drains
                    adde = nc.vector
                    adde2 = nc.gpsimd if j % 2 == 1 else nc.vector
                    t1bf = work.tile([128, I * 2 * H], BF16, tag="t1bf", name="t1bf")
                    for i in range(I):
                        tq = psumT1.tile([128, 1024], F32, tag="T1q", name="tq")
                        for h in range(2):
                            nc.tensor.matmul(
                                tq[:, h * 512 : h * 512 + H],
                                lhsT=lhs,
                                rhs=w2v[:, i * RP + h * H : i * RP + (h + 1) * H],
                                start=True,
                                stop=True,
                            )
                        nc.scalar.copy(
                            t1bf[:, i * RP : (i + 1) * RP].rearrange(
                                "p (q r) -> p q r", q=2
                            ),
                            tq[:, 0:1024].rearrange("p (q r) -> p q r", q=2)[
                                :, :, 0:H
                            ],
                        )

                    # P = T1*u (one op); a = sum_i P (pairwise adds; the lo
                    # pair always on Pool, the final add alternates Pool/DVE
                    # by j parity to balance the two engines)
                    nc.vector.tensor_mul(t1bf[:], t1bf[:], u_b[:])
                    pq = t1bf[:].rearrange("p (x n) -> p x n", x=4)
                    a02 = small.tile([128, 2 * RP], BF16, tag="a02", name="a02", bufs=2)
                    a2v = a02[:].rearrange("p (x n) -> p x n", x=2)
                    addf = nc.vector
                    nc.gpsimd.tensor_add(a2v[:, 0], pq[:, 0], pq[:, 2])
                    nc.vector.tensor_add(a2v[:, 1], pq[:, 1], pq[:, 3])
                    if t == 1:
                        a_cur = a1_v[:, j]
                        addf.tensor_add(a_cur, a2v[:, 0], a2v[:, 1])
                    else:
                        a_t = small.tile([128, RP], BF16, tag="a_t", name="a_t", bufs=2)
                        addf.tensor_add(a_t[:], a2v[:, 0], a2v[:, 1])
                        a_b = small.tile([128, RP], BF16, tag="a_b", name="a_b", bufs=2)
                        nc.vector.scalar_tensor_tensor(
                            a_b[:], a_t[:], g1_all[:, j : j + 1], a1_v[:, j],
                            OP.mult, OP.add,
                        )
                        a_cur = a_b[:]

                    eT[j] = a_cur

                def stageE(j, t=t, eT=eT):
                    a_cur = eT.pop(j)
                    # e = exp(a), Z accumulated for free
                    e_b = ework.tile([128, RP], BF16, tag="e_b", name="e_b")
                    nc.scalar.activation(
                        e_b[:], a_cur, ACT_F.Exp,
                        accum_out=zraw_all[:, j : j + 1],
                    )
                    e_rT = ework.tile([128, C128 * B], BF16, tag="e_rT", name="e_rT")
                    # alternate the two HWDGE queues so the ~1.7us per-transpose
                    # descriptor-gen doesn't serialize on one sequencer
                    dq = nc.sync if j % 2 == 0 else nc.scalar
                    dq.dma_start_transpose(
                        e_rT[:].rearrange("p (c b) -> p c b", c=C128), e_b[:]
                    )
                    eT[j] = e_rT

                def stageB(j, eT=eT, curs=curs, spss=spss):
                    e_rT = eT.pop(j)
                    cur = work.tile([128, C128 * I * B], BF16, tag="cur", name="cur")
                    curv = cur[:].rearrange("p (c i b) -> p c i b", c=C128, i=I)
                    nc.vector.tensor_mul(
                        curv,
                        e_rT[:]
                        .rearrange("p (c b) -> p c b", c=C128)
                        .unsqueeze(2)
                        .broadcast_to([128, C128, I, B]),
                        uTr_v,
                    )
                    sps = psumSV.tile([128, 512], F32, tag="sv", name="sps")
                    for c in range(C128):
                        for i in range(I):
                            nc.tensor.matmul(
                                sps[:, 0:O],
                                lhsT=curv[:, c, i],
                                rhs=wr_v[:, c, i, j],
                                start=(c == 0 and i == 0),
                                stop=(c == C128 - 1 and i == I - 1),
                            )
                    curs[j] = cur
                    spss[j] = sps

                def stageC(j, t=t, spss=spss):
                    sps = spss.pop(j)
                    s_all = s1_all if t == 1 else s2_all
                    sj = s_all[:, j * O : (j + 1) * O]
                    nc.scalar.copy(sj, sps[:, 0:O])
                    if t == 1:
                        vt = psumSV.tile([128, 512], F32, tag="sv", name="vt")
                        nc.tensor.transpose(vt[0:8, 0:128], sj, identity[:])
                        nc.scalar.copy(
                            sT_prev[:, j * 128 : (j + 1) * 128], vt[0:8, 0:128]
                        )

    # group-batched squash scalars; t=2 uses smaller groups so the
                    # output chunks (and their DMAs) finish before the tail
                    grp = GRP if t == 1 else 7
                    if j % grp == grp - 1:
                        g0 = j - grp + 1
                        sl = slice(g0, j + 1)
                        sqg = small.tile([128, grp * O], F32, tag="sqg", name="sqg", bufs=2)
                        nc.vector.tensor_mul(
                            sqg[:], s_all[:, g0 * O : (j + 1) * O],
                            s_all[:, g0 * O : (j + 1) * O],
                        )
                        nc.vector.reduce_sum(
                            m2_all[:, sl],
                            sqg[:].rearrange("p (j o) -> p j o", j=grp),
                            axis=AX.X,
                        )
                        zc = small.tile([128, grp], F32, tag="zc", name="zc")
                        nc.vector.tensor_scalar_add(
                            zc[:], zraw_all[:, sl], float(-NPAD)
                        )
                        nc.vector.reciprocal(zr_all[:, sl], zc[:])
                        q = small.tile([128, grp], F32, tag="q", name="q")
                        nc.vector.tensor_mul(q[:], zr_all[:, sl], zr_all[:, sl])
                        nc.vector.tensor_mul(q[:], q[:], m2_all[:, sl])
                        rt = small.tile([128, grp], F32, tag="rt", name="rt")
                        _dve_sqrt(nc, small, rt[:], q[:], grp, tag="isq")
                        den = small.tile([128, grp], F32, tag="den", name="den")
                        nc.vector.tensor_scalar_add(den[:], q[:], 1.0)
                        nc.vector.reciprocal(den[:], den[:])
                        gg = small.tile([128, grp], F32, tag="gg", name="gg")
                        nc.vector.tensor_mul(gg[:], rt[:], den[:])
                        nc.vector.tensor_mul(g_all[:, sl], gg[:], zr_all[:, sl])
                        if t == 2:
                            nc.vector.tensor_mul(
                                out_sb[:, g0 * O : (j + 1) * O].rearrange(
                                    "p (j o) -> p j o", j=grp
                                ),
                                s2_all[:, g0 * O : (j + 1) * O].rearrange(
                                    "p (j o) -> p j o", j=grp
                                ),
                                g_all[:, sl]
                                .unsqueeze(2)
                                .broadcast_to([128, grp, O]),
                            )
                            nc.gpsimd.dma_start(
                                d_out[:, g0 * O : (j + 1) * O],
                                out_sb[:, g0 * O : (j + 1) * O],
                            )

                # stageB runs two stages behind stageE so the exp ->
                # DMA-transpose -> cur chain (~4us latency) stays hidden
                for idx in range(JL + 4):
                    if idx < JL:
                        stageA(idx)
                    if 1 <= idx <= JL:
                        stageE(idx - 1)
                    if 3 <= idx <= JL + 2:
                        stageB(idx - 3)
                    if 4 <= idx:
                        stageC(idx - 4)


# ---------------------------------------------------------------------------
# Host side
# ---------------------------------------------------------------------------

_NC_CACHE = None


def _get_nc():
    global _NC_CACHE
    if _NC_CACHE is None:
        _NC_CACHE = build_nc()
    return _NC_CACHE


def _host_prep(u, W):
    bf = ml_dtypes.bfloat16
    u = np.ascontiguousarray(u, dtype=np.float32)
    Wq = np.ascontiguousarray(W.reshape(J, R, O, I), dtype=np.float32)

    up = np.zeros((B, RP, I), np.float32)
    up[:, :R] = u
    u_b = np.ascontiguousarray(up.transpose(0, 2, 1).reshape(128, I * RP).astype(bf))
    uTp = np.ascontiguousarray(up.transpose(1, 2, 0))
    uTr = np.ascontiguousarray(
        uTp.reshape(C128, 128, I * B).transpose(1, 0, 2).reshape(128, C128 * I * B).astype(bf)
    )

    in_maps = []
    for k in range(NCORES):
        j0 = k * JL
        Wk = np.zeros((JL, R, O, I), np.float32)
        real = min(JL, max(0, J - j0))
        if real > 0:
            Wk[:real] = Wq[j0 : j0 + real]
        Wkp = np.zeros((JL, RP, O, I), np.float32)
        Wkp[:, :R] = Wk

        wr = np.ascontiguousarray(
            Wkp.transpose(1, 3, 0, 2).reshape(C128, 128, I * JL * O)
            .transpose(1, 0, 2).reshape(128, C128 * I * JL * O).astype(bf)
        )
        w2 = np.ascontiguousarray(
            Wkp.transpose(0, 2, 3, 1).reshape(JL, 8, I * RP).astype(bf)
        )
        in_maps.append(
            {
                "Wr": wr,
                "W2": w2,
                "u_b": u_b,
                "uTr": uTr,
            }
        )
    return in_maps


def run_cores(u, W, trace=False):
    nc = _get_nc()
    in_maps = _host_prep(u, W)
    res = run_bass_kernel_spmd(
        nc, in_maps, core_ids=list(range(NCORES)), trace=trace
    )
    return res


def kernel(u, W):
    res = run_cores(u, W, trace=False)
    parts = []
    for k in range(NCORES):
        vk = res.results[k]["v_out"]
        parts.append(vk.reshape(B, JL, O))
    full = np.concatenate(parts, axis=1)[:, :J, :]
    return np.ascontiguousarray(full.astype(np.float32))

